# revision 5
# baseline (speedup 1.0000x reference)
"""Trainium2 Bass kernel for nn_Plasmodium_50027779064181 (sparse_attention).

Tensor-parallel over 4 heads x 2 query-row halves = 8 cores.
Core c: row-half = c//4, head = c%4, decode slice = rows [c*256, c*256+256).
Single NEFF launch; on-device collectives exchange the h activation,
k/v row-halves (within head pairs), and the pre-decode head concat.

Heavy matmuls run in float32r (TF32-like precision, full PE rate for
free dims >= 256).  All DMA/matmul addressing is identical on every core;
per-core identity enters only through input DATA (shards + the halfsel
indicator used to pick the real block out of the padded AllToAll).
"""

import numpy as np

import concourse.bass as bass
import concourse.mybir as mybir
import concourse.tile as tile_mod
from concourse.tile import TileContext
from concourse.vector_clock import ScopedClock

F32 = mybir.dt.float32
F32R = mybir.dt.float32r
AF = mybir.ActivationFunctionType
OP = mybir.AluOpType

N, S, L = 2048, 1024, 2048
P, D = 4, 512
NH = N // 2            # rows per half
NS = N // 8            # rows per decode slice
EPS = 1e-10
LN_EPS = 1e-5
SCALE = 1.0 / (float(D) ** 0.5)

N_CORES = 8

# ---------------------------------------------------------------- patches

_PATCHED = False


def _apply_patches():
    """walrus in this container rejects >1 semaphore wait per instruction;
    split Tile's multi-wait sync_infos into chains of single-wait NOPs."""
    global _PATCHED
    if _PATCHED:
        return
    _PATCHED = True

    _orig_lower = tile_mod.TileContext._lower_ordered_insts

    def _patched_lower(self, ordered):
        nc = self.nc
        for bb_name, insts in ordered.items():
            new_list = []
            for inst in insts:
                si = getattr(inst, "sync_info", None)
                if si is not None and si.on_wait and len(si.on_wait) > 1:
                    waits = list(si.on_wait)
                    for w in waits[:-1]:
                        new_list.append(
                            mybir.InstNoOp(
                                name=f"I-ws-{nc.next_id()}",
                                sync_info=mybir.SyncInfo(on_wait=[w], on_update=[]),
                                engine=inst.engine,
                                bass_nofuse=True,
                            )
                        )
                    inst.sync_info = mybir.SyncInfo(
                        on_wait=[waits[-1]], on_update=list(si.on_update or [])
                    )
                new_list.append(inst)
            insts[:] = new_list
        return _orig_lower(self, ordered)

    tile_mod.TileContext._lower_ordered_insts = _patched_lower

    def _patched_drain(self, tick_clock, wait_clock):
        nc = self.nc
        probe = nc.sync.nop()
        wait_clock.add_sem_waits(
            probe.ins, ScopedClock({None: tick_clock.global_clock})
        )
        si = probe.ins.sync_info
        waits = list(si.on_wait or []) if si else []
        probe.ins.sync_info = mybir.SyncInfo(on_wait=waits[:1], on_update=[])
        for w in waits[1:]:
            n2 = nc.sync.nop()
            n2.ins.sync_info = mybir.SyncInfo(on_wait=[w], on_update=[])
        nc.sync.drain()
        nc.all_engine_barrier()
        popped = nc._tile_sem_poison_stack.pop()
        assert popped is self._sem_poison
        nc.clear_and_free_semaphores(list(self.sems.allocated().values()))
        nc.all_engine_barrier()

    tile_mod.TileContext._drain_and_barrier = _patched_drain


# ---------------------------------------------------------------- build


def build_kernel():
    _apply_patches()
    nc = bass.Bass()

    # ---- per-core input shards (f32r where consumed by matmuls)
    xT = nc.dram_tensor("xT", [S, NH], F32R, kind="ExternalInput")    # x[half].T
    xTa = nc.dram_tensor("xTa", [S, NS], F32R, kind="ExternalInput")  # x[slice c].T
    We_d = nc.dram_tensor("We_d", [S, L], F32R, kind="ExternalInput")
    Wq_d = nc.dram_tensor("Wq_d", [S, D], F32R, kind="ExternalInput")
    Wk_d = nc.dram_tensor("Wk_d", [L, D], F32R, kind="ExternalInput")
    Wv_d = nc.dram_tensor("Wv_d", [L, D], F32R, kind="ExternalInput")
    Wg_d = nc.dram_tensor("Wg_d", [L, S], F32R, kind="ExternalInput")  # ln_g*Wd
    ones_d = nc.dram_tensor("ones_d", [128, 4], F32R, kind="ExternalInput")
    beT_d = nc.dram_tensor("beT_d", [128, 16], F32, kind="ExternalInput")
    bqT_d = nc.dram_tensor("bqT_d", [128, 4], F32, kind="ExternalInput")
    bkT_d = nc.dram_tensor("bkT_d", [128, 4], F32, kind="ExternalInput")
    bvT_d = nc.dram_tensor("bvT_d", [128, 4], F32, kind="ExternalInput")
    bv_row_d = nc.dram_tensor("bv_row_d", [1, D], F32, kind="ExternalInput")
    u_row_d = nc.dram_tensor("u_row_d", [1, S], F32, kind="ExternalInput")
    w0_row_d = nc.dram_tensor("w0_row_d", [1, S], F32, kind="ExternalInput")
    halfsel_d = nc.dram_tensor("halfsel_d", [128, 2], F32, kind="ExternalInput")
    consts_d = nc.dram_tensor("consts_d", [128, 2], F32, kind="ExternalInput")

    # ---- outputs
    out_part = nc.dram_tensor("out_part", [NS, S], F32, kind="ExternalOutput")
    corr_part = nc.dram_tensor("corr_part", [NH, N], F32, kind="ExternalOutput")

    # ---- internal DRAM
    hpart_d = nc.dram_tensor("hpart_d", [L, NS], F32R)
    g_h = nc.dram_tensor("g_h", [4, L, NS], F32R)
    kTh_d = nc.dram_tensor("kTh_d", [D, NH], F32R)
    vTh_d = nc.dram_tensor("vTh_d", [D, NH], F32R)
    vh_d = nc.dram_tensor("vh_d", [NH, D], F32R)
    g_k = nc.dram_tensor("g_k", [2, D, NH], F32R)
    g_vT = nc.dram_tensor("g_vT", [2, D, NH], F32R)
    g_v = nc.dram_tensor("g_v", [2, NH, D], F32R)
    a2a_in = nc.dram_tensor("a2a_in", [8, D, NS], F32R)
    a2a_out = nc.dram_tensor("a2a_out", [8, D, NS], F32R)
    rk_d = nc.dram_tensor("rk_d", [1, N], F32)
    rv_d = nc.dram_tensor("rv_d", [1, N], F32)
    rq_d = nc.dram_tensor("rq_d", [1, NH], F32)
    rkh_d = nc.dram_tensor("rkh_d", [1, NH], F32)
    rs_d = nc.dram_tensor("rs_d", [1, NH], F32)

    HALF_GROUPS = [[0, 1, 2, 3], [4, 5, 6, 7]]
    PAIR_GROUPS = [[0, 4], [1, 5], [2, 6], [3, 7]]
    ALL_GROUP = [[0, 1, 2, 3, 4, 5, 6, 7]]

    with TileContext(nc, num_cores=N_CORES) as tc:
        with tc.tile_pool(name="persist", bufs=1) as pp:
            ones_sb = pp.tile([128, 4], F32R, tag="ones")
            nc.sync.dma_start(ones_sb[:], ones_d[:])
            qT_sb = pp.tile([128, 4 * NH], F32R, tag="qT")
            kTh_sb = pp.tile([128, 4 * NH], F32R, tag="kTh")
            bvb_sb = pp.tile([128, D], F32, tag="bvb")
            nc.sync.dma_start(bvb_sb[:], bv_row_d[:].partition_broadcast(128))
            ub_sb = pp.tile([128, S], F32, tag="ub")
            nc.sync.dma_start(ub_sb[:], u_row_d[:].partition_broadcast(128))
            w0b_sb = pp.tile([128, S], F32, tag="w0b")
            nc.sync.dma_start(w0b_sb[:], w0_row_d[:].partition_broadcast(128))
            hsel_sb = pp.tile([128, 2], F32, tag="hsel")
            nc.sync.dma_start(hsel_sb[:], halfsel_d[:])
            cst_sb = pp.tile([128, 2], F32, tag="cst")
            nc.sync.dma_start(cst_sb[:], consts_d[:])

            # ============ stage A: hT slice, qT half ======================
            with (
                tc.tile_pool(name="poolA", bufs=1) as pa,
                tc.tile_pool(name="psA", bufs=2, space="PSUM") as psA,
                tc.tile_pool(name="psQ", bufs=2, space="PSUM") as psQ,
            ):
                We_sb = pa.tile([128, 8 * L], F32R, tag="We")
                for si in range(8):
                    nc.sync.dma_start(
                        We_sb[:, si * L:(si + 1) * L],
                        We_d[si * 128:(si + 1) * 128, :],
                    )
                xTh_sb = pa.tile([128, 8 * NH], F32R, tag="xTh")
                for si in range(8):
                    nc.sync.dma_start(
                        xTh_sb[:, si * NH:(si + 1) * NH],
                        xT[si * 128:(si + 1) * 128, :],
                    )
                xTa_sb = pa.tile([128, 8 * NS], F32R, tag="xTa")
                for si in range(8):
                    nc.sync.dma_start(
                        xTa_sb[:, si * NS:(si + 1) * NS],
                        xTa[si * 128:(si + 1) * 128, :],
                    )
                Wq_sb = pa.tile([128, 8 * D], F32R, tag="Wq")
                for si in range(8):
                    nc.sync.dma_start(
                        Wq_sb[:, si * D:(si + 1) * D],
                        Wq_d[si * 128:(si + 1) * 128, :],
                    )
                beT_sb = pa.tile([128, 16], F32, tag="beT")
                nc.sync.dma_start(beT_sb[:], beT_d[:])
                bqT_sb = pa.tile([128, 4], F32, tag="bqT")
                nc.sync.dma_start(bqT_sb[:], bqT_d[:])

                for li in range(16):
                    ps = psA.tile([128, NS], F32, tag="psA")
                    for si in range(8):
                        nc.tensor.matmul(
                            ps[:],
                            We_sb[:, si * L + li * 128: si * L + (li + 1) * 128],
                            xTa_sb[:, si * NS:(si + 1) * NS],
                            start=(si == 0), stop=(si == 7),
                        )
                    ht = pa.tile([128, NS], F32R, tag="ht")
                    nc.scalar.activation(ht[:], ps[:], AF.Tanh,
                                         bias=beT_sb[:, li:li + 1])
                    nc.sync.dma_start(hpart_d[li * 128:(li + 1) * 128, :], ht[:])

                nc.gpsimd.collective_compute(
                    "AllGather", OP.bypass, replica_groups=HALF_GROUPS,
                    ins=[hpart_d[:]], outs=[g_h[:]],
                )

                # qT for own half rows (overlaps the h gather)
                for ji in range(4):
                    for nb in range(2):
                        ps = psQ.tile([128, 512], F32, tag="psQ")
                        for si in range(8):
                            nc.tensor.matmul(
                                ps[:],
                                Wq_sb[:, si * D + ji * 128: si * D + (ji + 1) * 128],
                                xTh_sb[:, si * NH + nb * 512: si * NH + (nb + 1) * 512],
                                start=(si == 0), stop=(si == 7),
                            )
                        nc.scalar.activation(
                            qT_sb[:, ji * NH + nb * 512: ji * NH + (nb + 1) * 512],
                            ps[:], AF.Identity, bias=bqT_sb[:, ji:ji + 1],
                        )

            # ============ stage C: kT/vT/v halves =========================
            with (
                tc.tile_pool(name="poolC", bufs=1) as pc_,
                tc.tile_pool(name="psC", bufs=3, space="PSUM") as psC,
            ):
                hT_sb = pc_.tile([128, 16 * NH], F32R, tag="hT")
                for li in range(16):
                    for q in range(4):
                        nc.sync.dma_start(
                            hT_sb[:, li * NH + q * NS: li * NH + (q + 1) * NS],
                            g_h[q, li * 128:(li + 1) * 128, :],
                        )
                Wk_sb = pc_.tile([128, 16 * D], F32R, tag="Wk")
                for li in range(16):
                    nc.sync.dma_start(
                        Wk_sb[:, li * D:(li + 1) * D],
                        Wk_d[li * 128:(li + 1) * 128, :],
                    )
                Wv_sb = pc_.tile([128, 16 * D], F32R, tag="Wv")
                for li in range(16):
                    nc.sync.dma_start(
                        Wv_sb[:, li * D:(li + 1) * D],
                        Wv_d[li * 128:(li + 1) * 128, :],
                    )
                bkT_sb = pc_.tile([128, 4], F32, tag="bkT")
                nc.sync.dma_start(bkT_sb[:], bkT_d[:])
                bvT_sb = pc_.tile([128, 4], F32, tag="bvT")
                nc.sync.dma_start(bvT_sb[:], bvT_d[:])

                # kT half (kept in SBUF for corr) + DMA out for pair gather
                for ji in range(4):
                    for nb in range(2):
                        ps = psC.tile([128, 512], F32, tag="psC")
                        for li in range(16):
                            nc.tensor.matmul(
                                ps[:],
                                Wk_sb[:, li * D + ji * 128: li * D + (ji + 1) * 128],
                                hT_sb[:, li * NH + nb * 512: li * NH + (nb + 1) * 512],
                                start=(li == 0), stop=(li == 15),
                            )
                        dstk = kTh_sb[:, ji * NH + nb * 512: ji * NH + (nb + 1) * 512]
                        nc.scalar.activation(dstk, ps[:], AF.Identity,
                                             bias=bkT_sb[:, ji:ji + 1])
                        nc.sync.dma_start(
                            kTh_d[ji * 128:(ji + 1) * 128, nb * 512:(nb + 1) * 512],
                            dstk,
                        )
                # vT half (streamed)
                for ji in range(4):
                    for nb in range(2):
                        ps = psC.tile([128, 512], F32, tag="psC")
                        for li in range(16):
                            nc.tensor.matmul(
                                ps[:],
                                Wv_sb[:, li * D + ji * 128: li * D + (ji + 1) * 128],
                                hT_sb[:, li * NH + nb * 512: li * NH + (nb + 1) * 512],
                                start=(li == 0), stop=(li == 15),
                            )
                        ev = pc_.tile([128, 512], F32R, tag="evT")
                        nc.scalar.activation(ev[:], ps[:], AF.Identity,
                                             bias=bvT_sb[:, ji:ji + 1])
                        nc.sync.dma_start(
                            vTh_d[ji * 128:(ji + 1) * 128, nb * 512:(nb + 1) * 512],
                            ev[:],
                        )
                # v natural half (streamed)
                for ni in range(8):
                    ps = psC.tile([128, 512], F32, tag="psC")
                    for li in range(16):
                        nc.tensor.matmul(
                            ps[:],
                            hT_sb[:, li * NH + ni * 128: li * NH + (ni + 1) * 128],
                            Wv_sb[:, li * D:(li + 1) * D],
                            start=(li == 0), stop=(li == 15),
                        )
                    ev = pc_.tile([128, 512], F32R, tag="evV")
                    nc.vector.tensor_tensor(ev[:], ps[:], bvb_sb[:], OP.add)
                    nc.sync.dma_start(vh_d[ni * 128:(ni + 1) * 128, :], ev[:])

                nc.gpsimd.collective_compute(
                    "AllGather", OP.bypass, replica_groups=PAIR_GROUPS,
                    ins=[kTh_d[:]], outs=[g_k[:]],
                )
                nc.gpsimd.collective_compute(
                    "AllGather", OP.bypass, replica_groups=PAIR_GROUPS,
                    ins=[vTh_d[:]], outs=[g_vT[:]],
                )
                nc.gpsimd.collective_compute(
                    "AllGather", OP.bypass, replica_groups=PAIR_GROUPS,
                    ins=[vh_d[:]], outs=[g_v[:]],
                )

            # ============ attention + corr ================================
            with tc.tile_pool(name="poolT", bufs=1) as pt:
                kT_sb = pt.tile([128, 4 * N], F32R, tag="kT")
                vT_sb = pt.tile([128, 4 * N], F32R, tag="vT")
                for ji in range(4):
                    for hb in range(2):
                        nc.sync.dma_start(
                            kT_sb[:, ji * N + hb * NH: ji * N + (hb + 1) * NH],
                            g_k[hb, ji * 128:(ji + 1) * 128, :],
                        )
                        nc.sync.dma_start(
                            vT_sb[:, ji * N + hb * NH: ji * N + (hb + 1) * NH],
                            g_vT[hb, ji * 128:(ji + 1) * 128, :],
                        )
                v_sb = pt.tile([128, 16 * D], F32R, tag="v")
                for hb in range(2):
                    for mi in range(8):
                        nc.sync.dma_start(
                            v_sb[:, (hb * 8 + mi) * D:(hb * 8 + mi + 1) * D],
                            g_v[hb, mi * 128:(mi + 1) * 128, :],
                        )

                # ---- norms: rows 1/sqrt(sum sq) in free layout
                sq_sb = pt.tile([128, N], F32R, tag="sq")
                rowt = pt.tile([1, 512], F32, tag="rowt")
                with tc.tile_pool(name="psR", bufs=1, space="PSUM") as psR:
                    norm_jobs = (
                        (kT_sb, N, rk_d, 1.0),
                        (vT_sb, N, rv_d, 1.0),
                        (qT_sb, NH, rq_d, SCALE),
                        (kTh_sb, NH, rkh_d, 1.0),
                    )
                    for (src_sb, n_tot, row_out, scale_mul) in norm_jobs:
                        nblk = n_tot // 512
                        pss = []
                        for b in range(nblk):
                            psnb = psR.tile([4, 512], F32, tag=f"psR{b}",
                                            name=f"psRt{b}")
                            pss.append(psnb)
                        for ji in range(4):
                            nc.scalar.activation(
                                sq_sb[:, :n_tot],
                                src_sb[:, ji * n_tot:(ji + 1) * n_tot].bitcast(F32),
                                AF.Square,
                            )
                            for b in range(nblk):
                                nc.tensor.matmul(
                                    pss[b][:], ones_sb[:],
                                    sq_sb[:, b * 512:(b + 1) * 512],
                                    start=(ji == 0), stop=(ji == 3),
                                )
                        for b in range(nblk):
                            nc.scalar.activation(rowt[:], pss[b][0:1, :],
                                                 AF.Sqrt, bias=cst_sb[0:1, 0:1])
                            nc.vector.reciprocal(rowt[:], rowt[:])
                            if scale_mul != 1.0:
                                nc.vector.tensor_scalar_mul(
                                    rowt[:], rowt[:], scale_mul)
                            nc.sync.dma_start(
                                row_out[0:1, b * 512:(b + 1) * 512], rowt[:])

                rvb_sb = pt.tile([128, N], F32, tag="rvb")
                nc.sync.dma_start(rvb_sb[:], rv_d[:].partition_broadcast(128))
                rqb_sb = pt.tile([128, NH], F32, tag="rqb")
                nc.sync.dma_start(rqb_sb[:], rq_d[:].partition_broadcast(128))
                rk_sb = pt.tile([128, 16], F32, tag="rkp")
                nc.sync.dma_start(rk_sb[:], rk_d[0].rearrange("(b a) -> a b", a=128))
                rkh_sb = pt.tile([128, 8], F32, tag="rkh")
                nc.sync.dma_start(rkh_sb[:], rkh_d[0].rearrange("(b a) -> a b", a=128))

                # ---- scores -> pT -> attnT + sums, per 512-wide n block
                with (
                    tc.tile_pool(name="psS", bufs=3, space="PSUM") as psS,
                    tc.tile_pool(name="psA2", bufs=2, space="PSUM") as psA2,
                    tc.tile_pool(name="psSum", bufs=1, space="PSUM") as psSum,
                ):
                    for nb in range(2):
                        pT_sb = pt.tile([128, 16 * 512], F32R, tag="pT")
                        for mi in range(16):
                            ps_s = psS.tile([128, 512], F32, tag="psS")
                            for ji in range(4):
                                nc.tensor.matmul(
                                    ps_s[:],
                                    kT_sb[:, ji * N + mi * 128: ji * N + (mi + 1) * 128],
                                    qT_sb[:, ji * NH + nb * 512: ji * NH + (nb + 1) * 512],
                                    start=(ji == 0), stop=(ji == 3),
                                )
                            st = pt.tile([128, 512], F32, tag="st")
                            nc.vector.scalar_tensor_tensor(
                                st[:], ps_s[:], rk_sb[:, mi:mi + 1],
                                rqb_sb[:, nb * 512:(nb + 1) * 512],
                                OP.mult, OP.mult,
                            )
                            nc.scalar.activation(
                                pT_sb[:, mi * 512:(mi + 1) * 512], st[:], AF.Exp)
                        ps_sum = psSum.tile([4, 512], F32, tag="psSum")
                        for mi in range(16):
                            nc.tensor.matmul(
                                ps_sum[:], ones_sb[:],
                                pT_sb[:, mi * 512:(mi + 1) * 512],
                                start=(mi == 0), stop=(mi == 15),
                            )
                        rs_sb = pt.tile([1, 512], F32, tag="rs")
                        nc.vector.tensor_scalar_add(rs_sb[:], ps_sum[0:1, :], EPS)
                        nc.vector.reciprocal(rs_sb[:], rs_sb[:])
                        nc.sync.dma_start(rs_d[0:1, nb * 512:(nb + 1) * 512], rs_sb[:])
                        rsb_sb = pt.tile([128, 512], F32, tag="rsb")
                        nc.sync.dma_start(
                            rsb_sb[:],
                            rs_d[:, nb * 512:(nb + 1) * 512].partition_broadcast(128),
                        )
                        for ii in range(4):
                            ps_a = psA2.tile([128, 512], F32, tag="psA2")
                            for mi in range(16):
                                nc.tensor.matmul(
                                    ps_a[:],
                                    v_sb[:, mi * D + ii * 128: mi * D + (ii + 1) * 128],
                                    pT_sb[:, mi * 512:(mi + 1) * 512],
                                    start=(mi == 0), stop=(mi == 15),
                                )
                            aoT = pt.tile([128, 512], F32R, tag="aoT")
                            nc.vector.tensor_tensor(aoT[:], ps_a[:], rsb_sb[:], OP.mult)
                            # write each 256-col quarter into BOTH half slots;
                            # the receiver selects via halfsel.
                            for hh in range(2):
                                for qq in range(2):
                                    nc.sync.dma_start(
                                        a2a_in[4 * hh + 2 * nb + qq,
                                               ii * 128:(ii + 1) * 128, :],
                                        aoT[:, qq * NS:(qq + 1) * NS],
                                    )

                        # corr rows for this n block (own half rows)
                        for ni in range(4):
                            nig = nb * 4 + ni
                            for mb in range(4):
                                ps_c = psS.tile([128, 512], F32, tag="psS")
                                for ji in range(4):
                                    nc.tensor.matmul(
                                        ps_c[:],
                                        kTh_sb[:, ji * NH + nig * 128:
                                               ji * NH + (nig + 1) * 128],
                                        vT_sb[:, ji * N + mb * 512:
                                              ji * N + (mb + 1) * 512],
                                        start=(ji == 0), stop=(ji == 3),
                                    )
                                co = pt.tile([128, 512], F32, tag="co")
                                nc.vector.scalar_tensor_tensor(
                                    co[:], ps_c[:], rkh_sb[:, nig:nig + 1],
                                    rvb_sb[:, mb * 512:(mb + 1) * 512],
                                    OP.mult, OP.mult,
                                )
                                nc.sync.dma_start(
                                    corr_part[nig * 128:(nig + 1) * 128,
                                              mb * 512:(mb + 1) * 512],
                                    co[:],
                                )

                nc.gpsimd.collective_compute(
                    "AllToAll", OP.bypass, replica_groups=ALL_GROUP,
                    ins=[a2a_in[:]], outs=[a2a_out[:]],
                )

            # ============ LN + decode =====================================
            with (
                tc.tile_pool(name="poolD", bufs=1) as pd_,
                tc.tile_pool(name="psD", bufs=3, space="PSUM") as psD,
                tc.tile_pool(name="psLN", bufs=2, space="PSUM") as psLN,
            ):
                aT_sb = pd_.tile([128, 16 * NS], F32R, tag="aT")
                t0 = pd_.tile([128, NS], F32, tag="a2a_t0")
                t1 = pd_.tile([128, NS], F32, tag="a2a_t1")
                tmid = pd_.tile([128, NS], F32, tag="a2a_tm")
                for pg in range(4):
                    for lj in range(4):
                        nc.sync.dma_start(
                            t0[:], a2a_out[pg, lj * 128:(lj + 1) * 128, :].bitcast(F32))
                        nc.sync.dma_start(
                            t1[:], a2a_out[4 + pg, lj * 128:(lj + 1) * 128, :].bitcast(F32))
                        nc.vector.tensor_scalar_mul(tmid[:], t1[:], hsel_sb[:, 1:2])
                        nc.vector.scalar_tensor_tensor(
                            aT_sb[:, (pg * 4 + lj) * NS:(pg * 4 + lj + 1) * NS],
                            t0[:], hsel_sb[:, 0:1], tmid[:], OP.mult, OP.add,
                        )

                Wg_sb = pd_.tile([128, 16 * S], F32R, tag="Wg")
                for li in range(16):
                    nc.sync.dma_start(
                        Wg_sb[:, li * S:(li + 1) * S],
                        Wg_d[li * 128:(li + 1) * 128, :],
                    )

                a2_sb = pd_.tile([128, 16 * NS], F32R, tag="a2")
                for li in range(16):
                    nc.scalar.activation(
                        a2_sb[:, li * NS:(li + 1) * NS],
                        aT_sb[:, li * NS:(li + 1) * NS].bitcast(F32),
                        AF.Square,
                    )
                mu_sb = pd_.tile([128, 2], F32, tag="mu")
                s2_sb = pd_.tile([128, 2], F32, tag="s2")
                for ni in range(2):
                    psm = psLN.tile([128, 4], F32, tag="psm")
                    for li in range(16):
                        nc.tensor.matmul(
                            psm[:],
                            aT_sb[:, li * NS + ni * 128: li * NS + (ni + 1) * 128],
                            ones_sb[:],
                            start=(li == 0), stop=(li == 15),
                        )
                    nc.scalar.mul(mu_sb[:, ni:ni + 1], psm[:, 0:1], 1.0 / L)
                    psv = psLN.tile([128, 4], F32, tag="psv")
                    for li in range(16):
                        nc.tensor.matmul(
                            psv[:],
                            a2_sb[:, li * NS + ni * 128: li * NS + (ni + 1) * 128],
                            ones_sb[:],
                            start=(li == 0), stop=(li == 15),
                        )
                    nc.scalar.mul(s2_sb[:, ni:ni + 1], psv[:, 0:1], 1.0 / L)
                musq = pd_.tile([128, 2], F32, tag="musq")
                nc.vector.tensor_mul(musq[:], mu_sb[:], mu_sb[:])
                var_sb = pd_.tile([128, 2], F32, tag="var")
                nc.vector.tensor_sub(var_sb[:], s2_sb[:], musq[:])
                inv_sb = pd_.tile([128, 2], F32, tag="inv")
                nc.scalar.activation(inv_sb[:], var_sb[:], AF.Sqrt, bias=cst_sb[:, 1:2])
                nc.vector.reciprocal(inv_sb[:], inv_sb[:])
                nminv = pd_.tile([128, 2], F32, tag="nminv")
                nc.vector.tensor_mul(nminv[:], mu_sb[:], inv_sb[:])
                nc.vector.tensor_scalar_mul(nminv[:], nminv[:], -1.0)

                for ni in range(2):
                    for tb in range(2):
                        ps_z = psD.tile([128, 512], F32, tag="psD")
                        for li in range(16):
                            nc.tensor.matmul(
                                ps_z[:],
                                aT_sb[:, li * NS + ni * 128: li * NS + (ni + 1) * 128],
                                Wg_sb[:, li * S + tb * 512: li * S + (tb + 1) * 512],
                                start=(li == 0), stop=(li == 15),
                            )
                        z1 = pd_.tile([128, 512], F32, tag="z1")
                        nc.vector.tensor_scalar_mul(z1[:], ps_z[:],
                                                    inv_sb[:, ni:ni + 1])
                        z2 = pd_.tile([128, 512], F32, tag="z2")
                        nc.vector.scalar_tensor_tensor(
                            z2[:], ub_sb[:, tb * 512:(tb + 1) * 512],
                            nminv[:, ni:ni + 1], z1[:], OP.mult, OP.add,
                        )
                        nc.vector.tensor_tensor(
                            z2[:], z2[:], w0b_sb[:, tb * 512:(tb + 1) * 512], OP.add)
                        o_sb = pd_.tile([128, 512], F32, tag="osb")
                        nc.scalar.activation(o_sb[:], z2[:], AF.Tanh)
                        nc.sync.dma_start(
                            out_part[ni * 128:(ni + 1) * 128,
                                     tb * 512:(tb + 1) * 512],
                            o_sb[:],
                        )

    return nc


# ---------------------------------------------------------------- host


_CACHE = {}


def _run(in_maps, **kw):
    from concourse.bass_utils import run_bass_kernel_spmd

    if "nc" not in _CACHE:
        _CACHE["nc"] = build_kernel()
    return run_bass_kernel_spmd(_CACHE["nc"], in_maps, list(range(N_CORES)), **kw)


def make_in_maps(x, We, be, Wq, bq, Wk, bk, Wv, bv, ln_g, ln_b, Wd, bd):
    x = np.asarray(x, np.float32)
    We = np.asarray(We, np.float32); be = np.asarray(be, np.float32)
    Wq = np.asarray(Wq, np.float32); bq = np.asarray(bq, np.float32)
    Wk = np.asarray(Wk, np.float32); bk = np.asarray(bk, np.float32)
    Wv = np.asarray(Wv, np.float32); bv = np.asarray(bv, np.float32)
    ln_g = np.asarray(ln_g, np.float32); ln_b = np.asarray(ln_b, np.float32)
    Wd = np.asarray(Wd, np.float32); bd = np.asarray(bd, np.float32)

    Wg = (ln_g[:, None] * Wd).astype(np.float32)
    u_row = Wg.sum(axis=0, dtype=np.float64).astype(np.float32)[None, :]
    w0_row = (ln_b @ Wd + bd).astype(np.float32)[None, :]
    beT = be.reshape(16, 128).T.copy()
    ones = np.ones((128, 4), np.float32)
    consts = np.tile(np.array([[EPS, LN_EPS]], np.float32), (128, 1))
    xT_full = np.ascontiguousarray(x.T)

    in_maps = []
    for c in range(N_CORES):
        half, p = c // 4, c % 4
        hd = slice(p * D, (p + 1) * D)
        halfsel = np.zeros((128, 2), np.float32)
        halfsel[:, half] = 1.0
        in_maps.append({
            "xT": np.ascontiguousarray(xT_full[:, half * NH:(half + 1) * NH]),
            "xTa": np.ascontiguousarray(xT_full[:, c * NS:(c + 1) * NS]),
            "We_d": We,
            "Wq_d": np.ascontiguousarray(Wq[:, hd]),
            "Wk_d": np.ascontiguousarray(Wk[:, hd]),
            "Wv_d": np.ascontiguousarray(Wv[:, hd]),
            "Wg_d": Wg,
            "ones_d": ones,
            "beT_d": beT,
            "bqT_d": bq[hd].reshape(4, 128).T.copy(),
            "bkT_d": bk[hd].reshape(4, 128).T.copy(),
            "bvT_d": bv[hd].reshape(4, 128).T.copy(),
            "bv_row_d": bv[hd][None, :].copy(),
            "u_row_d": u_row,
            "w0_row_d": w0_row,
            "halfsel_d": halfsel,
            "consts_d": consts,
        })
    return in_maps


def assemble(results):
    out = np.empty((N, S), np.float32)
    corr = np.empty((P, N, N), np.float32)
    for c in range(N_CORES):
        half, p = c // 4, c % 4
        out[c * NS:(c + 1) * NS, :] = results[c]["out_part"]
        corr[p, half * NH:(half + 1) * NH, :] = results[c]["corr_part"]
    return out, corr


def kernel(x, We, be, Wq, bq, Wk, bk, Wv, bv, ln_g, ln_b, Wd, bd, **_kw):
    in_maps = make_in_maps(x, We, be, Wq, bq, Wk, bk, Wv, bv,
                           ln_g, ln_b, Wd, bd)
    res = _run(in_maps)
    _CACHE["last_result"] = res
    return assemble(res.results)
